# revision 42
# baseline (speedup 1.0000x reference)
"""Trainium2 Bass kernel for the AlpacaMH head.

Math (per sample b, per (z,u) pair, A = Linv[b,z,u], 128x128):
    phi = MLP_encoder(x[b])                       # (P,)
    w_zu = A^T phi
    sigma_raw[zu] = w_zu . phi   (= phi^T A phi)
    mu[zu]        = w_zu . q_zu  (= phi^T A q)
    cov[zu] = exp(logSigEps[u]) * (1 + sigma_raw[zu])

Strategy: pure data-parallel over batch across 8 NeuronCores (8 samples
per core).  The 256 MiB Linv tensor is streamed once from HBM in its
NATURAL CONTIGUOUS layout (16 KB per partition -> line-rate DMA
descriptors), cast f32->bf16 inside the DMA (gpsimd SWDGE ring; 1/4 of
chunks ride the SP HWDGE ring as f32 + ACT convert).

The key trick: no on-chip repartition/transpose of Linv at all.  In the
flat layout a chunk holds 32 matrices; partition p carries rows
i = (p%4)*32 + c (c = 0..31) of matrix zu = p//4.  For each column
block c the chunk block tf[:, 128c:128c+128] is the matmul STATIONARY
operand, and the moving operand is the 32-column sparse matrix
R_c[p, n] = (p//4 == n) * phi[(p%4)*32 + c]; accumulating the 32 blocks
in PSUM yields W[j, zu] = A_zu^T phi for all 32 matrices: PE cost is
only (64 ldw + 32 mm) cycles per block.  W is evacuated (tiny: 32 KB
per sample), then one DVE multiply pair (W*phi | W*qT) and two
ones-matmuls reduce to the (sigma, mu) columns per sample.
DMA of Linv is the roofline (~33.5 MB/core @ ~358 GB/s => ~94 us).
"""

import numpy as np

import concourse.bass as bass  # noqa: F401  (registers engine classes)
import concourse.mybir as mybir
import concourse.tile as tile
from concourse import bacc
from concourse.masks import make_identity

F32 = mybir.dt.float32
BF16 = mybir.dt.bfloat16
AF = mybir.ActivationFunctionType
ALU = mybir.AluOpType

# Problem dims (hardcoded per spec)
B, Z, U, P, X, H = 64, 8, 8, 128, 64, 512
NCORES = 8
BS = B // NCORES          # samples per core
ZU = Z * U                # 64 (z,u) pairs per sample
CZU = 32                  # zu pairs per flat chunk (2 MB f32)
NCHUNK = ZU // CZU        # chunks per sample (2)
NBLK = CZU                # column blocks per chunk (32)


def build_nc():
    nc = bacc.Bacc(None, target_bir_lowering=False, debug=False)
    with tile.TileContext(nc) as tc:
        with (
            tc.tile_pool(name="dram", bufs=1, space="DRAM") as dram,
            tc.tile_pool(name="const", bufs=1) as const,
            tc.tile_pool(name="wts", bufs=1) as wts,
            tc.tile_pool(name="flat", bufs=8) as flatp,
            tc.tile_pool(name="flatf", bufs=2) as flatfp,
            tc.tile_pool(name="small", bufs=2) as small,
            tc.tile_pool(name="wps", bufs=3, space="PSUM") as wpsp,
            tc.tile_pool(name="encps", bufs=2, space="PSUM") as encps,
            tc.tile_pool(name="qps", bufs=2, space="PSUM") as qps,
            tc.tile_pool(name="rps", bufs=1, space="PSUM") as rps,
        ):
            # ---- DRAM parameters (names must match in_maps keys) ----
            x_d = dram.tile([BS, X], F32, kind="ExternalInput", name="x", uniquify=False)
            linv_d = dram.tile([BS, Z, U, P, P], F32, kind="ExternalInput", name="Linv", uniquify=False)
            q_d = dram.tile([BS, Z, U, 1, P], F32, kind="ExternalInput", name="Q", uniquify=False)
            w1_d = dram.tile([X, H], F32, kind="ExternalInput", name="W1", uniquify=False)
            b1_d = dram.tile([H], F32, kind="ExternalInput", name="b1", uniquify=False)
            w2_d = dram.tile([H, H], F32, kind="ExternalInput", name="W2", uniquify=False)
            b2_d = dram.tile([H], F32, kind="ExternalInput", name="b2", uniquify=False)
            w3_d = dram.tile([H, H], F32, kind="ExternalInput", name="W3", uniquify=False)
            b3_d = dram.tile([H], F32, kind="ExternalInput", name="b3", uniquify=False)
            w4_d = dram.tile([H, P], F32, kind="ExternalInput", name="W4", uniquify=False)
            b4_d = dram.tile([P], F32, kind="ExternalInput", name="b4", uniquify=False)
            ls_d = dram.tile([U], F32, kind="ExternalInput", name="logSigEps", uniquify=False)
            mu_d = dram.tile([BS, Z, U, 1], F32, kind="ExternalOutput", name="mu_out", uniquify=False)
            cov_d = dram.tile([BS, Z, U], F32, kind="ExternalOutput", name="cov_out", uniquify=False)

            # ---- constants ----
            ident = const.tile([128, 128], F32)
            make_identity(nc, ident[:])
            ones_bf = const.tile([128, 1], BF16)
            nc.vector.memset(ones_bf[:], 1.0)
            # mask[p, n] = (p//4 == n), [128, 32] bf16
            mask = const.tile([128, NBLK], BF16)
            nc.gpsimd.memset(mask[:], 1.0)
            nc.gpsimd.affine_select(out=mask[:], in_=mask[:], compare_op=ALU.is_ge,
                                    fill=0.0, base=0, channel_multiplier=1,
                                    pattern=[[-4, NBLK]])
            nc.gpsimd.affine_select(out=mask[:], in_=mask[:], compare_op=ALU.is_ge,
                                    fill=0.0, base=3, channel_multiplier=-1,
                                    pattern=[[4, NBLK]])
            # E[p4, g, r] = (r == p4): [4, 128] bf16 viewed [4, 32, 4]
            e_rep = const.tile([4, 128], BF16)
            nc.gpsimd.memset(e_rep[:], 0.0)
            nc.gpsimd.affine_select(
                out=e_rep[:].rearrange("x (g r) -> x g r", r=4),
                in_=e_rep[:].rearrange("x (g r) -> x g r", r=4),
                compare_op=ALU.not_equal, fill=1.0, base=0,
                channel_multiplier=-1, pattern=[[0, 32], [1, 4]])

            # ---- head loads (sync ring, consumption order) ----
            x_sb = small.tile([BS, X], F32, tag="x_sb")
            nc.sync.dma_start(out=x_sb[:], in_=x_d[:])
            brow = wts.tile([4, 3, 128], F32)
            nc.sync.dma_start(out=brow[:, 0, :], in_=b1_d[:].rearrange("(c p) -> c p", c=4))
            nc.sync.dma_start(out=brow[:, 1, :], in_=b2_d[:].rearrange("(c p) -> c p", c=4))
            nc.sync.dma_start(out=brow[:, 2, :], in_=b3_d[:].rearrange("(c p) -> c p", c=4))
            b4row = wts.tile([1, P], F32)
            nc.sync.dma_start(out=b4row[:], in_=b4_d[None, :])
            qall = wts.tile([ZU, BS, P], F32)
            nc.sync.dma_start(out=qall[:], in_=q_d[:].rearrange("s z u one j -> (z u) s (one j)"))
            w1_sb = wts.tile([X, H], F32)
            nc.sync.dma_start(out=w1_sb[:], in_=w1_d[:])
            w4_sb = wts.tile([128, 4, P], F32)
            nc.sync.dma_start(out=w4_sb[:], in_=w4_d[:].rearrange("(kk p) h -> p kk h", p=128))
            w2_sb = wts.tile([128, 4, H], F32)
            nc.sync.dma_start(out=w2_sb[:], in_=w2_d[:].rearrange("(kk p) h -> p kk h", p=128))
            w3_sb = wts.tile([128, 4, H], F32)
            nc.sync.dma_start(out=w3_sb[:], in_=w3_d[:].rearrange("(kk p) h -> p kk h", p=128))
            # sigfac64[zu] = exp(logSigEps[zu % 8]), exact f32 path
            lsfull = wts.tile([ZU, 1], F32)
            for z in range(Z):
                nc.sync.dma_start(out=lsfull[z * U:(z + 1) * U, :], in_=ls_d[:, None])
            sigfac64 = const.tile([ZU, 1], F32)
            nc.scalar.activation(sigfac64[:], lsfull[:], AF.Exp)

            lvf = linv_d[:].rearrange("s z u i j -> s (z u i j)")

            # ---------- Linv chunk loads (software-pipelined lookahead) ----
            # gpsimd SWDGE ring casts f32->bf16 in flight; every 4th chunk
            # rides the SP HWDGE ring as f32 (converted on ACT just-in-time).
            # The first gpsimd DMA is gated on w3 (last weight) via a dummy
            # WAW dep so the encoder weight loads get the SDMA engines first.
            LOOKAHEAD = 8
            chunk_tiles = {}

            def load_chunk(g):
                if g in chunk_tiles:
                    return
                s, h = divmod(g, NCHUNK)
                tfb = flatp.tile([128, NCHUNK, CZU * P], BF16, tag="tf",
                                 bufs=4, name=f"tfb_{s}")
                if g == 0:
                    nc.vector.tensor_copy(tfb[0:1, 0:1, 0:1], w3_sb[0:1, 0:1, 0:1])
                nc.gpsimd.dma_start(
                    out=tfb[:],
                    in_=lvf[s, :].rearrange("(h p f) -> p h f", h=NCHUNK, p=128))
                chunk_tiles[s * NCHUNK] = (tfb[:, 0, :], False)
                chunk_tiles[s * NCHUNK + 1] = (tfb[:, 1, :], False)

            for g in range(LOOKAHEAD):
                load_chunk(g)

            # ---------- encoder ----------
            bcols = wts.tile([128, 3, 4], F32)
            bt_ps = qps.tile([128, 16], F32, tag="qt", name="bt_ps")
            for l in range(3):
                nc.tensor.transpose(bt_ps[:, l * 4:l * 4 + 4], brow[:, l, :], ident[:4, :4])
            nc.scalar.copy(bcols[:], bt_ps[:, :12].rearrange("p (l c) -> p l c", l=3))
            b4_ps = qps.tile([128, 16], F32, tag="qt", name="b4_ps")
            nc.tensor.transpose(b4_ps[:, 0:1], b4row[:], ident[:1, :1])
            b4_sb = wts.tile([P, 1], F32)
            nc.scalar.copy(b4_sb[:], b4_ps[:, 0:1])

            xt_ps = encps.tile([X, BS], F32, tag="t", name="xt_ps")
            nc.tensor.transpose(xt_ps[:], x_sb[:], ident[:BS, :BS])
            xt = small.tile([X, BS], F32, tag="xt")
            nc.scalar.copy(xt[:], xt_ps[:])

            def elu_into(pre_ps, bias_col, out_ap):
                # out = elu(pre + bias) = max(v, exp(min(v,0)) - 1)
                v = small.tile([128, BS], F32, tag="elu_v")
                nc.vector.tensor_scalar_add(v[:], pre_ps[:], bias_col)
                m = small.tile([128, BS], F32, tag="elu_m")
                nc.vector.tensor_scalar_min(m[:], v[:], 0.0)
                e = small.tile([128, BS], F32, tag="elu_e")
                nc.scalar.activation(e[:], m[:], AF.Exp)
                nc.vector.tensor_scalar_add(e[:], e[:], -1.0)
                nc.vector.tensor_tensor(out_ap, v[:], e[:], op=ALU.max)

            h1 = small.tile([128, 4, BS], F32, tag="h1")
            for m in range(4):
                ps = encps.tile([128, BS], F32, tag="t", name=f"eps1_{m}")
                nc.tensor.matmul(ps[:], w1_sb[:, m * 128:(m + 1) * 128], xt[:],
                                 start=True, stop=True)
                elu_into(ps, bcols[:, 0, m:m + 1], h1[:, m, :])
            h2 = small.tile([128, 4, BS], F32, tag="h2")
            for m in range(4):
                ps = encps.tile([128, BS], F32, tag="t", name=f"eps2_{m}")
                for kk in range(4):
                    nc.tensor.matmul(ps[:], w2_sb[:, kk, m * 128:(m + 1) * 128],
                                     h1[:, kk, :], start=(kk == 0), stop=(kk == 3))
                elu_into(ps, bcols[:, 1, m:m + 1], h2[:, m, :])
            h3 = small.tile([128, 4, BS], F32, tag="h3")
            for m in range(4):
                ps = encps.tile([128, BS], F32, tag="t", name=f"eps3_{m}")
                for kk in range(4):
                    nc.tensor.matmul(ps[:], w3_sb[:, kk, m * 128:(m + 1) * 128],
                                     h2[:, kk, :], start=(kk == 0), stop=(kk == 3))
                elu_into(ps, bcols[:, 2, m:m + 1], h3[:, m, :])

            phi_ps = encps.tile([P, BS], F32, tag="t", name="phi_ps")
            for kk in range(4):
                nc.tensor.matmul(phi_ps[:], w4_sb[:, kk, :], h3[:, kk, :],
                                 start=(kk == 0), stop=(kk == 3))
            phiT = small.tile([P, BS], F32, tag="phiT")
            nc.vector.tensor_scalar_add(phiT[:], phi_ps[:], b4_sb[:])
            phiT_bf = const.tile([P, BS], BF16)
            nc.vector.tensor_copy(phiT_bf[:], phiT[:])

            # phirows [s, i] for phi-pattern construction
            pr_ps = encps.tile([BS, P], F32, tag="t", name="pr_ps")
            nc.tensor.transpose(pr_ps[:], phiT[:], ident[:])
            phirows = small.tile([BS, P], F32, tag="phirows")
            nc.scalar.copy(phirows[:], pr_ps[:])
            # G[p4, s, c] = phi_s[p4*32 + c]  (4 tiny SBUF->SBUF DMAs on the
            # Act HWDGE ring -- NOT gpsimd, whose queue must keep streaming
            # Linv without waiting on the encoder)
            g_f = wts.tile([4, BS, NBLK], F32)
            for p4 in range(4):
                nc.scalar.dma_start(out=g_f[p4:p4 + 1, :, :],
                                    in_=phirows[:, p4 * NBLK:(p4 + 1) * NBLK])
            g_bf = wts.tile([4, BS, NBLK], BF16)
            nc.vector.tensor_copy(g_bf[:], g_f[:])
            # phipat[p, s, c] = phi_s[(p%4)*32 + c] = E^T @ G
            pp_ps = qps.tile([128, BS * NBLK], F32, tag="qt", name="pp_ps")
            nc.tensor.matmul(pp_ps[:], e_rep[:], g_bf[:].rearrange("x s c -> x (s c)"),
                             start=True, stop=True)
            phipat = small.tile([128, BS, NBLK], BF16, tag="phipat")
            nc.scalar.copy(phipat[:], pp_ps[:].rearrange("p (s c) -> p s c", s=BS))

            # ---------- per-sample prep: R tiles + Q^T ----------
            r_tiles = []
            r32_tiles = {}
            qt_tiles = []
            for s in range(BS):
                rt = const.tile([128, NBLK, NBLK], BF16, name=f"r_{s}")
                nc.vector.tensor_tensor(
                    rt[:],
                    phipat[:, s, :][:, :, None].broadcast_to([128, NBLK, NBLK]),
                    mask[:, None, :].broadcast_to([128, NBLK, NBLK]),
                    op=ALU.mult)
                r_tiles.append(rt)
                qt_ps = qps.tile([P, ZU], F32, tag="qt", name=f"qt_ps_{s}")
                nc.tensor.transpose(qt_ps[:], qall[:, s, :], ident[:ZU, :ZU])
                qt = small.tile([P, ZU], BF16, tag="qt_sb", bufs=8, name=f"qt_{s}")
                nc.scalar.copy(qt[:], qt_ps[:])
                qt_tiles.append(qt)

            # ---------- main chunk pipeline ----------
            mscols = const.tile([ZU, BS, 2], F32)   # [zu, s, (sig|mu)]

            for s in range(BS):
                wall = small.tile([P, ZU], BF16, tag="wall", bufs=3, name=f"wall_{s}")
                for h in range(NCHUNK):
                    g = s * NCHUNK + h
                    if g + LOOKAHEAD < BS * NCHUNK:
                        load_chunk(g + LOOKAHEAD)
                    tf, is_f32 = chunk_tiles[g]
                    rh = r32_tiles[s] if is_f32 else r_tiles[s]
                    wp = wpsp.tile([P, CZU], F32, tag="w", name=f"wp_{g}")
                    for c in range(NBLK):
                        nc.tensor.matmul(wp[:], tf[:, c * 128:(c + 1) * 128],
                                         rh[:, c, :],
                                         start=(c == 0), stop=(c == NBLK - 1))
                    if h % 2 == 0:
                        nc.vector.tensor_copy(wall[:, h * CZU:(h + 1) * CZU], wp[:])
                    else:
                        nc.scalar.copy(wall[:, h * CZU:(h + 1) * CZU], wp[:])

                # pr = [ W*phi | W*qT ] -> ones-matmuls -> sig, mu columns
                pr = small.tile([P, 2 * ZU], BF16, tag="pr", name=f"pr_{s}")
                nc.vector.tensor_tensor(
                    pr[:, 0:ZU], wall[:],
                    phiT_bf[:, s:s + 1].broadcast_to([P, ZU]), op=ALU.mult)
                nc.vector.tensor_tensor(pr[:, ZU:2 * ZU], wall[:], qt_tiles[s][:],
                                        op=ALU.mult)
                ms_ps = rps.tile([ZU, 2], F32, tag="rows", name=f"ms_{s}")
                nc.tensor.matmul(ms_ps[:, 0:1], pr[:, 0:ZU], ones_bf[:],
                                 start=True, stop=True)
                nc.tensor.matmul(ms_ps[:, 1:2], pr[:, ZU:2 * ZU], ones_bf[:],
                                 start=True, stop=True)
                nc.scalar.copy(mscols[:, s, :], ms_ps[:])

            # ---------- finalize ----------
            cov64 = const.tile([ZU, BS], F32)
            nc.vector.tensor_scalar_add(cov64[:], mscols[:, :, 0], 1.0)
            nc.vector.tensor_scalar_mul(cov64[:], cov64[:], sigfac64[:])
            mu_t_ps = encps.tile([BS, ZU], F32, tag="t", name="mu_t_ps")
            nc.tensor.transpose(mu_t_ps[:], mscols[:, :, 1], ident[:ZU, :ZU])
            mu_t = small.tile([BS, ZU], F32, tag="mu_t")
            nc.scalar.copy(mu_t[:], mu_t_ps[:])
            nc.sync.dma_start(out=mu_d[:].rearrange("s z u one -> s (z u one)"), in_=mu_t[:])
            cov_t_ps = encps.tile([BS, ZU], F32, tag="t", name="cov_t_ps")
            nc.tensor.transpose(cov_t_ps[:], cov64[:], ident[:ZU, :ZU])
            cov_t = small.tile([BS, ZU], F32, tag="cov_t")
            nc.scalar.copy(cov_t[:], cov_t_ps[:])
            nc.sync.dma_start(out=cov_d[:].rearrange("s z u -> s (z u)"), in_=cov_t[:])

    nc.compile()
    return nc


_NC = None


def _get_nc():
    global _NC
    if _NC is None:
        _NC = build_nc()
    return _NC


def _in_maps(inputs):
    f = {k: np.ascontiguousarray(np.asarray(v, dtype=np.float32)) for k, v in inputs.items()}
    maps = []
    for c in range(NCORES):
        sl = slice(c * BS, (c + 1) * BS)
        maps.append({
            "x": f["x"][sl],
            "Linv": f["Linv"][sl],
            "Q": f["Q"][sl],
            "W1": f["W1"], "b1": f["b1"],
            "W2": f["W2"], "b2": f["b2"],
            "W3": f["W3"], "b3": f["b3"],
            "W4": f["W4"], "b4": f["b4"],
            "logSigEps": f["logSigEps"],
        })
    return maps


def kernel(**inputs):
    from concourse.bass_utils import run_bass_kernel_spmd

    nc = _get_nc()
    maps = _in_maps(inputs)
    res = run_bass_kernel_spmd(nc, maps, core_ids=list(range(NCORES)))
    mu = np.concatenate([np.asarray(res.results[i]["mu_out"]) for i in range(NCORES)], axis=0)
    cov = np.concatenate([np.asarray(res.results[i]["cov_out"]) for i in range(NCORES)], axis=0)
    return mu.astype(np.float32), cov.astype(np.float32)


# revision 43
# speedup vs baseline: 1.2065x; 1.2065x over previous
"""Trainium2 Bass kernel for the AlpacaMH head.

Math (per sample b, per (z,u) pair, A = Linv[b,z,u], 128x128):
    phi = MLP_encoder(x[b])                       # (P,)
    w_zu = A^T phi
    sigma_raw[zu] = w_zu . phi   (= phi^T A phi)
    mu[zu]        = w_zu . q_zu  (= phi^T A q)
    cov[zu] = exp(logSigEps[u]) * (1 + sigma_raw[zu])

Strategy: pure data-parallel over batch across 8 NeuronCores (8 samples
per core).  The 256 MiB Linv tensor is streamed once from HBM in its
NATURAL CONTIGUOUS layout (16 KB per partition -> line-rate DMA
descriptors), cast f32->bf16 inside the DMA (gpsimd SWDGE ring; 1/4 of
chunks ride the SP HWDGE ring as f32 + ACT convert).

The key trick: no on-chip repartition/transpose of Linv at all.  In the
flat layout a chunk holds 32 matrices; partition p carries rows
i = (p%4)*32 + c (c = 0..31) of matrix zu = p//4.  For each column
block c the chunk block tf[:, 128c:128c+128] is the matmul STATIONARY
operand, and the moving operand is the 32-column sparse matrix
R_c[p, n] = (p//4 == n) * phi[(p%4)*32 + c]; accumulating the 32 blocks
in PSUM yields W[j, zu] = A_zu^T phi for all 32 matrices: PE cost is
only (64 ldw + 32 mm) cycles per block.  W is evacuated (tiny: 32 KB
per sample), then one DVE multiply pair (W*phi | W*qT) and two
ones-matmuls reduce to the (sigma, mu) columns per sample.
DMA of Linv is the roofline (~33.5 MB/core @ ~358 GB/s => ~94 us).
"""

import numpy as np

import concourse.bass as bass  # noqa: F401  (registers engine classes)
import concourse.mybir as mybir
import concourse.tile as tile
from concourse import bacc
from concourse.masks import make_identity

F32 = mybir.dt.float32
BF16 = mybir.dt.bfloat16
AF = mybir.ActivationFunctionType
ALU = mybir.AluOpType

# Problem dims (hardcoded per spec)
B, Z, U, P, X, H = 64, 8, 8, 128, 64, 512
NCORES = 8
BS = B // NCORES          # samples per core
ZU = Z * U                # 64 (z,u) pairs per sample
CZU = 32                  # zu pairs per flat chunk (2 MB f32)
NCHUNK = ZU // CZU        # chunks per sample (2)
NBLK = CZU                # column blocks per chunk (32)


def build_nc():
    nc = bacc.Bacc(None, target_bir_lowering=False, debug=False)
    with tile.TileContext(nc) as tc:
        with (
            tc.tile_pool(name="dram", bufs=1, space="DRAM") as dram,
            tc.tile_pool(name="const", bufs=1) as const,
            tc.tile_pool(name="wts", bufs=1) as wts,
            tc.tile_pool(name="flat", bufs=8) as flatp,
            tc.tile_pool(name="flatf", bufs=2) as flatfp,
            tc.tile_pool(name="small", bufs=2) as small,
            tc.tile_pool(name="wps", bufs=3, space="PSUM") as wpsp,
            tc.tile_pool(name="encps", bufs=2, space="PSUM") as encps,
            tc.tile_pool(name="qps", bufs=2, space="PSUM") as qps,
            tc.tile_pool(name="rps", bufs=1, space="PSUM") as rps,
        ):
            # ---- DRAM parameters (names must match in_maps keys) ----
            x_d = dram.tile([BS, X], F32, kind="ExternalInput", name="x", uniquify=False)
            linv_d = dram.tile([BS, Z, U, P, P], F32, kind="ExternalInput", name="Linv", uniquify=False)
            q_d = dram.tile([BS, Z, U, 1, P], F32, kind="ExternalInput", name="Q", uniquify=False)
            w1_d = dram.tile([X, H], F32, kind="ExternalInput", name="W1", uniquify=False)
            b1_d = dram.tile([H], F32, kind="ExternalInput", name="b1", uniquify=False)
            w2_d = dram.tile([H, H], F32, kind="ExternalInput", name="W2", uniquify=False)
            b2_d = dram.tile([H], F32, kind="ExternalInput", name="b2", uniquify=False)
            w3_d = dram.tile([H, H], F32, kind="ExternalInput", name="W3", uniquify=False)
            b3_d = dram.tile([H], F32, kind="ExternalInput", name="b3", uniquify=False)
            w4_d = dram.tile([H, P], F32, kind="ExternalInput", name="W4", uniquify=False)
            b4_d = dram.tile([P], F32, kind="ExternalInput", name="b4", uniquify=False)
            ls_d = dram.tile([U], F32, kind="ExternalInput", name="logSigEps", uniquify=False)
            mu_d = dram.tile([BS, Z, U, 1], F32, kind="ExternalOutput", name="mu_out", uniquify=False)
            cov_d = dram.tile([BS, Z, U], F32, kind="ExternalOutput", name="cov_out", uniquify=False)

            # ---- constants ----
            ident = const.tile([128, 128], F32)
            make_identity(nc, ident[:])
            ones_bf = const.tile([128, 1], BF16)
            nc.vector.memset(ones_bf[:], 1.0)
            # mask[p, n] = (p//4 == n), [128, 32] bf16
            mask = const.tile([128, NBLK], BF16)
            nc.gpsimd.memset(mask[:], 1.0)
            nc.gpsimd.affine_select(out=mask[:], in_=mask[:], compare_op=ALU.is_ge,
                                    fill=0.0, base=0, channel_multiplier=1,
                                    pattern=[[-4, NBLK]])
            nc.gpsimd.affine_select(out=mask[:], in_=mask[:], compare_op=ALU.is_ge,
                                    fill=0.0, base=3, channel_multiplier=-1,
                                    pattern=[[4, NBLK]])
            # E[p4, g, r] = (r == p4): [4, 128] bf16 viewed [4, 32, 4]
            e_rep = const.tile([4, 128], BF16)
            nc.gpsimd.memset(e_rep[:], 0.0)
            nc.gpsimd.affine_select(
                out=e_rep[:].rearrange("x (g r) -> x g r", r=4),
                in_=e_rep[:].rearrange("x (g r) -> x g r", r=4),
                compare_op=ALU.not_equal, fill=1.0, base=0,
                channel_multiplier=-1, pattern=[[0, 32], [1, 4]])

            # ---- head loads (sync ring, consumption order) ----
            x_sb = small.tile([BS, X], F32, tag="x_sb")
            nc.sync.dma_start(out=x_sb[:], in_=x_d[:])
            brow = wts.tile([4, 3, 128], F32)
            nc.sync.dma_start(out=brow[:, 0, :], in_=b1_d[:].rearrange("(c p) -> c p", c=4))
            nc.sync.dma_start(out=brow[:, 1, :], in_=b2_d[:].rearrange("(c p) -> c p", c=4))
            nc.sync.dma_start(out=brow[:, 2, :], in_=b3_d[:].rearrange("(c p) -> c p", c=4))
            b4row = wts.tile([1, P], F32)
            nc.sync.dma_start(out=b4row[:], in_=b4_d[None, :])
            qall = wts.tile([ZU, BS, P], F32)
            nc.sync.dma_start(out=qall[:], in_=q_d[:].rearrange("s z u one j -> (z u) s (one j)"))
            w1_sb = wts.tile([X, H], F32)
            nc.sync.dma_start(out=w1_sb[:], in_=w1_d[:])
            w4_sb = wts.tile([128, 4, P], F32)
            nc.sync.dma_start(out=w4_sb[:], in_=w4_d[:].rearrange("(kk p) h -> p kk h", p=128))
            w2_sb = wts.tile([128, 4, H], F32)
            nc.sync.dma_start(out=w2_sb[:], in_=w2_d[:].rearrange("(kk p) h -> p kk h", p=128))
            w3_sb = wts.tile([128, 4, H], F32)
            nc.sync.dma_start(out=w3_sb[:], in_=w3_d[:].rearrange("(kk p) h -> p kk h", p=128))
            # sigfac64[zu] = exp(logSigEps[zu % 8]), exact f32 path
            lsfull = wts.tile([ZU, 1], F32)
            for z in range(Z):
                nc.sync.dma_start(out=lsfull[z * U:(z + 1) * U, :], in_=ls_d[:, None])
            sigfac64 = const.tile([ZU, 1], F32)
            nc.scalar.activation(sigfac64[:], lsfull[:], AF.Exp)

            lvf = linv_d[:].rearrange("s z u i j -> s (z u i j)")

            # ---------- Linv chunk loads (software-pipelined lookahead) ----
            # gpsimd SWDGE ring casts f32->bf16 in flight; every 4th chunk
            # rides the SP HWDGE ring as f32 (converted on ACT just-in-time).
            # The first gpsimd DMA is gated on w3 (last weight) via a dummy
            # WAW dep so the encoder weight loads get the SDMA engines first.
            LOOKAHEAD = 12
            chunk_tiles = {}

            def load_chunk(g):
                if g in chunk_tiles:
                    return
                s, h = divmod(g, NCHUNK)
                tfb = flatp.tile([128, NCHUNK, CZU * P], BF16, tag="tf",
                                 bufs=6, name=f"tfb_{s}")
                if g == 0:
                    nc.vector.tensor_copy(tfb[0:1, 0:1, 0:1], w3_sb[0:1, 0:1, 0:1])
                nc.gpsimd.dma_start(
                    out=tfb[:],
                    in_=lvf[s, :].rearrange("(h p f) -> p h f", h=NCHUNK, p=128))
                chunk_tiles[s * NCHUNK] = (tfb[:, 0, :], False)
                chunk_tiles[s * NCHUNK + 1] = (tfb[:, 1, :], False)

            for g in range(LOOKAHEAD):
                load_chunk(g)

            # ---------- encoder ----------
            bcols = wts.tile([128, 3, 4], F32)
            bt_ps = qps.tile([128, 16], F32, tag="qt", name="bt_ps")
            for l in range(3):
                nc.tensor.transpose(bt_ps[:, l * 4:l * 4 + 4], brow[:, l, :], ident[:4, :4])
            nc.scalar.copy(bcols[:], bt_ps[:, :12].rearrange("p (l c) -> p l c", l=3))
            b4_ps = qps.tile([128, 16], F32, tag="qt", name="b4_ps")
            nc.tensor.transpose(b4_ps[:, 0:1], b4row[:], ident[:1, :1])
            b4_sb = wts.tile([P, 1], F32)
            nc.scalar.copy(b4_sb[:], b4_ps[:, 0:1])

            xt_ps = encps.tile([X, BS], F32, tag="t", name="xt_ps")
            nc.tensor.transpose(xt_ps[:], x_sb[:], ident[:BS, :BS])
            xt = small.tile([X, BS], F32, tag="xt")
            nc.scalar.copy(xt[:], xt_ps[:])

            def elu_into(pre_ps, bias_col, out_ap):
                # out = elu(pre + bias) = max(v, exp(min(v,0)) - 1)
                v = small.tile([128, BS], F32, tag="elu_v")
                nc.vector.tensor_scalar_add(v[:], pre_ps[:], bias_col)
                m = small.tile([128, BS], F32, tag="elu_m")
                nc.vector.tensor_scalar_min(m[:], v[:], 0.0)
                e = small.tile([128, BS], F32, tag="elu_e")
                nc.scalar.activation(e[:], m[:], AF.Exp)
                nc.vector.tensor_scalar_add(e[:], e[:], -1.0)
                nc.vector.tensor_tensor(out_ap, v[:], e[:], op=ALU.max)

            h1 = small.tile([128, 4, BS], F32, tag="h1")
            for m in range(4):
                ps = encps.tile([128, BS], F32, tag="t", name=f"eps1_{m}")
                nc.tensor.matmul(ps[:], w1_sb[:, m * 128:(m + 1) * 128], xt[:],
                                 start=True, stop=True)
                elu_into(ps, bcols[:, 0, m:m + 1], h1[:, m, :])
            h2 = small.tile([128, 4, BS], F32, tag="h2")
            for m in range(4):
                ps = encps.tile([128, BS], F32, tag="t", name=f"eps2_{m}")
                for kk in range(4):
                    nc.tensor.matmul(ps[:], w2_sb[:, kk, m * 128:(m + 1) * 128],
                                     h1[:, kk, :], start=(kk == 0), stop=(kk == 3))
                elu_into(ps, bcols[:, 1, m:m + 1], h2[:, m, :])
            h3 = small.tile([128, 4, BS], F32, tag="h3")
            for m in range(4):
                ps = encps.tile([128, BS], F32, tag="t", name=f"eps3_{m}")
                for kk in range(4):
                    nc.tensor.matmul(ps[:], w3_sb[:, kk, m * 128:(m + 1) * 128],
                                     h2[:, kk, :], start=(kk == 0), stop=(kk == 3))
                elu_into(ps, bcols[:, 2, m:m + 1], h3[:, m, :])

            phi_ps = encps.tile([P, BS], F32, tag="t", name="phi_ps")
            for kk in range(4):
                nc.tensor.matmul(phi_ps[:], w4_sb[:, kk, :], h3[:, kk, :],
                                 start=(kk == 0), stop=(kk == 3))
            phiT = small.tile([P, BS], F32, tag="phiT")
            nc.vector.tensor_scalar_add(phiT[:], phi_ps[:], b4_sb[:])
            phiT_bf = const.tile([P, BS], BF16)
            nc.vector.tensor_copy(phiT_bf[:], phiT[:])

            # phirows [s, i] for phi-pattern construction
            pr_ps = encps.tile([BS, P], F32, tag="t", name="pr_ps")
            nc.tensor.transpose(pr_ps[:], phiT[:], ident[:])
            phirows = small.tile([BS, P], F32, tag="phirows")
            nc.scalar.copy(phirows[:], pr_ps[:])
            # G[p4, s, c] = phi_s[p4*32 + c]  (4 tiny SBUF->SBUF DMAs on the
            # Act HWDGE ring -- NOT gpsimd, whose queue must keep streaming
            # Linv without waiting on the encoder)
            g_f = wts.tile([4, BS, NBLK], F32)
            for p4 in range(4):
                nc.scalar.dma_start(out=g_f[p4:p4 + 1, :, :],
                                    in_=phirows[:, p4 * NBLK:(p4 + 1) * NBLK])
            g_bf = wts.tile([4, BS, NBLK], BF16)
            nc.vector.tensor_copy(g_bf[:], g_f[:])
            # phipat[p, s, c] = phi_s[(p%4)*32 + c] = E^T @ G
            pp_ps = qps.tile([128, BS * NBLK], F32, tag="qt", name="pp_ps")
            nc.tensor.matmul(pp_ps[:], e_rep[:], g_bf[:].rearrange("x s c -> x (s c)"),
                             start=True, stop=True)
            phipat = small.tile([128, BS, NBLK], BF16, tag="phipat")
            nc.scalar.copy(phipat[:], pp_ps[:].rearrange("p (s c) -> p s c", s=BS))

            # ---------- per-sample prep: R tiles + Q^T ----------
            r_tiles = []
            r32_tiles = {}
            qt_tiles = []
            for s in range(BS):
                rt = const.tile([128, NBLK, NBLK], BF16, name=f"r_{s}")
                nc.vector.tensor_tensor(
                    rt[:],
                    phipat[:, s, :][:, :, None].broadcast_to([128, NBLK, NBLK]),
                    mask[:, None, :].broadcast_to([128, NBLK, NBLK]),
                    op=ALU.mult)
                r_tiles.append(rt)
                qt_ps = qps.tile([P, ZU], F32, tag="qt", name=f"qt_ps_{s}")
                nc.tensor.transpose(qt_ps[:], qall[:, s, :], ident[:ZU, :ZU])
                qt = small.tile([P, ZU], BF16, tag="qt_sb", bufs=8, name=f"qt_{s}")
                nc.scalar.copy(qt[:], qt_ps[:])
                qt_tiles.append(qt)

            # ---------- main chunk pipeline ----------
            mscols = const.tile([ZU, BS, 2], F32)   # [zu, s, (sig|mu)]

            for s in range(BS):
                wall = small.tile([P, ZU], BF16, tag="wall", bufs=3, name=f"wall_{s}")
                for h in range(NCHUNK):
                    g = s * NCHUNK + h
                    if g + LOOKAHEAD < BS * NCHUNK:
                        load_chunk(g + LOOKAHEAD)
                    tf, is_f32 = chunk_tiles[g]
                    rh = r32_tiles[s] if is_f32 else r_tiles[s]
                    wp = wpsp.tile([P, CZU], F32, tag="w", name=f"wp_{g}")
                    for c in range(NBLK):
                        nc.tensor.matmul(wp[:], tf[:, c * 128:(c + 1) * 128],
                                         rh[:, c, :],
                                         start=(c == 0), stop=(c == NBLK - 1))
                    if h % 2 == 0:
                        nc.vector.tensor_copy(wall[:, h * CZU:(h + 1) * CZU], wp[:])
                    else:
                        nc.scalar.copy(wall[:, h * CZU:(h + 1) * CZU], wp[:])

                # pr = [ W*phi | W*qT ] -> ones-matmuls -> sig, mu columns
                pr = small.tile([P, 2 * ZU], BF16, tag="pr", name=f"pr_{s}")
                nc.vector.tensor_tensor(
                    pr[:, 0:ZU], wall[:],
                    phiT_bf[:, s:s + 1].broadcast_to([P, ZU]), op=ALU.mult)
                nc.vector.tensor_tensor(pr[:, ZU:2 * ZU], wall[:], qt_tiles[s][:],
                                        op=ALU.mult)
                ms_ps = rps.tile([ZU, 2], F32, tag="rows", name=f"ms_{s}")
                nc.tensor.matmul(ms_ps[:, 0:1], pr[:, 0:ZU], ones_bf[:],
                                 start=True, stop=True)
                nc.tensor.matmul(ms_ps[:, 1:2], pr[:, ZU:2 * ZU], ones_bf[:],
                                 start=True, stop=True)
                nc.scalar.copy(mscols[:, s, :], ms_ps[:])

            # ---------- finalize ----------
            cov64 = const.tile([ZU, BS], F32)
            nc.vector.tensor_scalar_add(cov64[:], mscols[:, :, 0], 1.0)
            nc.vector.tensor_scalar_mul(cov64[:], cov64[:], sigfac64[:])
            mu_t_ps = encps.tile([BS, ZU], F32, tag="t", name="mu_t_ps")
            nc.tensor.transpose(mu_t_ps[:], mscols[:, :, 1], ident[:ZU, :ZU])
            mu_t = small.tile([BS, ZU], F32, tag="mu_t")
            nc.scalar.copy(mu_t[:], mu_t_ps[:])
            nc.sync.dma_start(out=mu_d[:].rearrange("s z u one -> s (z u one)"), in_=mu_t[:])
            cov_t_ps = encps.tile([BS, ZU], F32, tag="t", name="cov_t_ps")
            nc.tensor.transpose(cov_t_ps[:], cov64[:], ident[:ZU, :ZU])
            cov_t = small.tile([BS, ZU], F32, tag="cov_t")
            nc.scalar.copy(cov_t[:], cov_t_ps[:])
            nc.sync.dma_start(out=cov_d[:].rearrange("s z u -> s (z u)"), in_=cov_t[:])

    nc.compile()
    return nc


_NC = None


def _get_nc():
    global _NC
    if _NC is None:
        _NC = build_nc()
    return _NC


def _in_maps(inputs):
    f = {k: np.ascontiguousarray(np.asarray(v, dtype=np.float32)) for k, v in inputs.items()}
    maps = []
    for c in range(NCORES):
        sl = slice(c * BS, (c + 1) * BS)
        maps.append({
            "x": f["x"][sl],
            "Linv": f["Linv"][sl],
            "Q": f["Q"][sl],
            "W1": f["W1"], "b1": f["b1"],
            "W2": f["W2"], "b2": f["b2"],
            "W3": f["W3"], "b3": f["b3"],
            "W4": f["W4"], "b4": f["b4"],
            "logSigEps": f["logSigEps"],
        })
    return maps


def kernel(**inputs):
    from concourse.bass_utils import run_bass_kernel_spmd

    nc = _get_nc()
    maps = _in_maps(inputs)
    res = run_bass_kernel_spmd(nc, maps, core_ids=list(range(NCORES)))
    mu = np.concatenate([np.asarray(res.results[i]["mu_out"]) for i in range(NCORES)], axis=0)
    cov = np.concatenate([np.asarray(res.results[i]["cov_out"]) for i in range(NCORES)], axis=0)
    return mu.astype(np.float32), cov.astype(np.float32)


# revision 44
# speedup vs baseline: 1.2192x; 1.0105x over previous
"""Trainium2 Bass kernel for the AlpacaMH head.

Math (per sample b, per (z,u) pair, A = Linv[b,z,u], 128x128):
    phi = MLP_encoder(x[b])                       # (P,)
    w_zu = A^T phi
    sigma_raw[zu] = w_zu . phi   (= phi^T A phi)
    mu[zu]        = w_zu . q_zu  (= phi^T A q)
    cov[zu] = exp(logSigEps[u]) * (1 + sigma_raw[zu])

Strategy: pure data-parallel over batch across 8 NeuronCores (8 samples
per core).  The 256 MiB Linv tensor is streamed once from HBM in its
NATURAL CONTIGUOUS layout (16 KB per partition -> line-rate DMA
descriptors), cast f32->bf16 inside the DMA (gpsimd SWDGE ring; 1/4 of
chunks ride the SP HWDGE ring as f32 + ACT convert).

The key trick: no on-chip repartition/transpose of Linv at all.  In the
flat layout a chunk holds 32 matrices; partition p carries rows
i = (p%4)*32 + c (c = 0..31) of matrix zu = p//4.  For each column
block c the chunk block tf[:, 128c:128c+128] is the matmul STATIONARY
operand, and the moving operand is the 32-column sparse matrix
R_c[p, n] = (p//4 == n) * phi[(p%4)*32 + c]; accumulating the 32 blocks
in PSUM yields W[j, zu] = A_zu^T phi for all 32 matrices: PE cost is
only (64 ldw + 32 mm) cycles per block.  W is evacuated (tiny: 32 KB
per sample), then one DVE multiply pair (W*phi | W*qT) and two
ones-matmuls reduce to the (sigma, mu) columns per sample.
DMA of Linv is the roofline (~33.5 MB/core @ ~358 GB/s => ~94 us).
"""

import numpy as np

import concourse.bass as bass  # noqa: F401  (registers engine classes)
import concourse.mybir as mybir
import concourse.tile as tile
from concourse import bacc
from concourse.masks import make_identity

F32 = mybir.dt.float32
BF16 = mybir.dt.bfloat16
AF = mybir.ActivationFunctionType
ALU = mybir.AluOpType

# Problem dims (hardcoded per spec)
B, Z, U, P, X, H = 64, 8, 8, 128, 64, 512
NCORES = 8
BS = B // NCORES          # samples per core
ZU = Z * U                # 64 (z,u) pairs per sample
CZU = 32                  # zu pairs per flat chunk (2 MB f32)
NCHUNK = ZU // CZU        # chunks per sample (2)
NBLK = CZU                # column blocks per chunk (32)


def build_nc():
    nc = bacc.Bacc(None, target_bir_lowering=False, debug=False)
    with tile.TileContext(nc) as tc:
        with (
            tc.tile_pool(name="dram", bufs=1, space="DRAM") as dram,
            tc.tile_pool(name="const", bufs=1) as const,
            tc.tile_pool(name="wts", bufs=1) as wts,
            tc.tile_pool(name="flat", bufs=8) as flatp,
            tc.tile_pool(name="flatf", bufs=2) as flatfp,
            tc.tile_pool(name="small", bufs=2) as small,
            tc.tile_pool(name="wps", bufs=3, space="PSUM") as wpsp,
            tc.tile_pool(name="encps", bufs=2, space="PSUM") as encps,
            tc.tile_pool(name="qps", bufs=2, space="PSUM") as qps,
            tc.tile_pool(name="rps", bufs=1, space="PSUM") as rps,
        ):
            # ---- DRAM parameters (names must match in_maps keys) ----
            x_d = dram.tile([BS, X], F32, kind="ExternalInput", name="x", uniquify=False)
            linv_d = dram.tile([BS, Z, U, P, P], F32, kind="ExternalInput", name="Linv", uniquify=False)
            q_d = dram.tile([BS, Z, U, 1, P], F32, kind="ExternalInput", name="Q", uniquify=False)
            w1_d = dram.tile([X, H], F32, kind="ExternalInput", name="W1", uniquify=False)
            b1_d = dram.tile([H], F32, kind="ExternalInput", name="b1", uniquify=False)
            w2_d = dram.tile([H, H], F32, kind="ExternalInput", name="W2", uniquify=False)
            b2_d = dram.tile([H], F32, kind="ExternalInput", name="b2", uniquify=False)
            w3_d = dram.tile([H, H], F32, kind="ExternalInput", name="W3", uniquify=False)
            b3_d = dram.tile([H], F32, kind="ExternalInput", name="b3", uniquify=False)
            w4_d = dram.tile([H, P], F32, kind="ExternalInput", name="W4", uniquify=False)
            b4_d = dram.tile([P], F32, kind="ExternalInput", name="b4", uniquify=False)
            ls_d = dram.tile([U], F32, kind="ExternalInput", name="logSigEps", uniquify=False)
            mu_d = dram.tile([BS, Z, U, 1], F32, kind="ExternalOutput", name="mu_out", uniquify=False)
            cov_d = dram.tile([BS, Z, U], F32, kind="ExternalOutput", name="cov_out", uniquify=False)

            # ---- constants ----
            ident = const.tile([128, 128], F32)
            make_identity(nc, ident[:])
            ones_bf = const.tile([128, 1], BF16)
            nc.vector.memset(ones_bf[:], 1.0)
            # mask[p, n] = (p//4 == n), [128, 32] bf16
            mask = const.tile([128, NBLK], BF16)
            nc.gpsimd.memset(mask[:], 1.0)
            nc.gpsimd.affine_select(out=mask[:], in_=mask[:], compare_op=ALU.is_ge,
                                    fill=0.0, base=0, channel_multiplier=1,
                                    pattern=[[-4, NBLK]])
            nc.gpsimd.affine_select(out=mask[:], in_=mask[:], compare_op=ALU.is_ge,
                                    fill=0.0, base=3, channel_multiplier=-1,
                                    pattern=[[4, NBLK]])
            # E[p4, g, r] = (r == p4): [4, 128] bf16 viewed [4, 32, 4]
            e_rep = const.tile([4, 128], BF16)
            nc.gpsimd.memset(e_rep[:], 0.0)
            nc.gpsimd.affine_select(
                out=e_rep[:].rearrange("x (g r) -> x g r", r=4),
                in_=e_rep[:].rearrange("x (g r) -> x g r", r=4),
                compare_op=ALU.not_equal, fill=1.0, base=0,
                channel_multiplier=-1, pattern=[[0, 32], [1, 4]])

            # ---- head loads (sync ring, consumption order) ----
            x_sb = small.tile([BS, X], F32, tag="x_sb")
            nc.sync.dma_start(out=x_sb[:], in_=x_d[:])
            brow = wts.tile([4, 3, 128], F32)
            nc.sync.dma_start(out=brow[:, 0, :], in_=b1_d[:].rearrange("(c p) -> c p", c=4))
            nc.sync.dma_start(out=brow[:, 1, :], in_=b2_d[:].rearrange("(c p) -> c p", c=4))
            nc.sync.dma_start(out=brow[:, 2, :], in_=b3_d[:].rearrange("(c p) -> c p", c=4))
            b4row = wts.tile([1, P], F32)
            nc.sync.dma_start(out=b4row[:], in_=b4_d[None, :])
            qall = wts.tile([ZU, BS, P], F32)
            nc.sync.dma_start(out=qall[:], in_=q_d[:].rearrange("s z u one j -> (z u) s (one j)"))
            w1_sb = wts.tile([X, H], F32)
            nc.sync.dma_start(out=w1_sb[:], in_=w1_d[:])
            w4_sb = wts.tile([128, 4, P], F32)
            nc.sync.dma_start(out=w4_sb[:], in_=w4_d[:].rearrange("(kk p) h -> p kk h", p=128))
            w2_sb = wts.tile([128, 4, H], F32)
            nc.sync.dma_start(out=w2_sb[:], in_=w2_d[:].rearrange("(kk p) h -> p kk h", p=128))
            w3_sb = wts.tile([128, 4, H], F32)
            nc.sync.dma_start(out=w3_sb[:], in_=w3_d[:].rearrange("(kk p) h -> p kk h", p=128))
            # sigfac64[zu] = exp(logSigEps[zu % 8]), exact f32 path
            lsfull = wts.tile([ZU, 1], F32)
            for z in range(Z):
                nc.sync.dma_start(out=lsfull[z * U:(z + 1) * U, :], in_=ls_d[:, None])
            sigfac64 = const.tile([ZU, 1], F32)
            nc.scalar.activation(sigfac64[:], lsfull[:], AF.Exp)

            lvf = linv_d[:].rearrange("s z u i j -> s (z u i j)")

            # ---------- Linv chunk loads (software-pipelined lookahead) ----
            # gpsimd SWDGE ring casts f32->bf16 in flight; every 4th chunk
            # rides the SP HWDGE ring as f32 (converted on ACT just-in-time).
            # The first gpsimd DMA is gated on w3 (last weight) via a dummy
            # WAW dep so the encoder weight loads get the SDMA engines first.
            LOOKAHEAD = 12
            chunk_tiles = {}

            def load_chunk(g):
                if g in chunk_tiles:
                    return
                s, h = divmod(g, NCHUNK)
                tfb = flatp.tile([128, NCHUNK, CZU * P], BF16, tag="tf",
                                 bufs=6, name=f"tfb_{s}")
                if g == 0:
                    nc.vector.tensor_copy(tfb[0:1, 0:1, 0:1], w2_sb[0:1, 0:1, 0:1])
                nc.gpsimd.dma_start(
                    out=tfb[:],
                    in_=lvf[s, :].rearrange("(h p f) -> p h f", h=NCHUNK, p=128))
                chunk_tiles[s * NCHUNK] = (tfb[:, 0, :], False)
                chunk_tiles[s * NCHUNK + 1] = (tfb[:, 1, :], False)

            for g in range(LOOKAHEAD):
                load_chunk(g)

            # ---------- encoder ----------
            bcols = wts.tile([128, 3, 4], F32)
            bt_ps = qps.tile([128, 16], F32, tag="qt", name="bt_ps")
            for l in range(3):
                nc.tensor.transpose(bt_ps[:, l * 4:l * 4 + 4], brow[:, l, :], ident[:4, :4])
            nc.scalar.copy(bcols[:], bt_ps[:, :12].rearrange("p (l c) -> p l c", l=3))
            b4_ps = qps.tile([128, 16], F32, tag="qt", name="b4_ps")
            nc.tensor.transpose(b4_ps[:, 0:1], b4row[:], ident[:1, :1])
            b4_sb = wts.tile([P, 1], F32)
            nc.scalar.copy(b4_sb[:], b4_ps[:, 0:1])

            xt_ps = encps.tile([X, BS], F32, tag="t", name="xt_ps")
            nc.tensor.transpose(xt_ps[:], x_sb[:], ident[:BS, :BS])
            xt = small.tile([X, BS], F32, tag="xt")
            nc.scalar.copy(xt[:], xt_ps[:])

            def elu_into(pre_ps, bias_col, out_ap):
                # out = elu(pre + bias) = max(v, exp(min(v,0)) - 1)
                v = small.tile([128, BS], F32, tag="elu_v")
                nc.vector.tensor_scalar_add(v[:], pre_ps[:], bias_col)
                m = small.tile([128, BS], F32, tag="elu_m")
                nc.vector.tensor_scalar_min(m[:], v[:], 0.0)
                e = small.tile([128, BS], F32, tag="elu_e")
                nc.scalar.activation(e[:], m[:], AF.Exp)
                nc.vector.tensor_scalar_add(e[:], e[:], -1.0)
                nc.vector.tensor_tensor(out_ap, v[:], e[:], op=ALU.max)

            h1 = small.tile([128, 4, BS], F32, tag="h1")
            for m in range(4):
                ps = encps.tile([128, BS], F32, tag="t", name=f"eps1_{m}")
                nc.tensor.matmul(ps[:], w1_sb[:, m * 128:(m + 1) * 128], xt[:],
                                 start=True, stop=True)
                elu_into(ps, bcols[:, 0, m:m + 1], h1[:, m, :])
            h2 = small.tile([128, 4, BS], F32, tag="h2")
            for m in range(4):
                ps = encps.tile([128, BS], F32, tag="t", name=f"eps2_{m}")
                for kk in range(4):
                    nc.tensor.matmul(ps[:], w2_sb[:, kk, m * 128:(m + 1) * 128],
                                     h1[:, kk, :], start=(kk == 0), stop=(kk == 3))
                elu_into(ps, bcols[:, 1, m:m + 1], h2[:, m, :])
            h3 = small.tile([128, 4, BS], F32, tag="h3")
            for m in range(4):
                ps = encps.tile([128, BS], F32, tag="t", name=f"eps3_{m}")
                for kk in range(4):
                    nc.tensor.matmul(ps[:], w3_sb[:, kk, m * 128:(m + 1) * 128],
                                     h2[:, kk, :], start=(kk == 0), stop=(kk == 3))
                elu_into(ps, bcols[:, 2, m:m + 1], h3[:, m, :])

            phi_ps = encps.tile([P, BS], F32, tag="t", name="phi_ps")
            for kk in range(4):
                nc.tensor.matmul(phi_ps[:], w4_sb[:, kk, :], h3[:, kk, :],
                                 start=(kk == 0), stop=(kk == 3))
            phiT = small.tile([P, BS], F32, tag="phiT")
            nc.vector.tensor_scalar_add(phiT[:], phi_ps[:], b4_sb[:])
            phiT_bf = const.tile([P, BS], BF16)
            nc.vector.tensor_copy(phiT_bf[:], phiT[:])

            # phirows [s, i] for phi-pattern construction
            pr_ps = encps.tile([BS, P], F32, tag="t", name="pr_ps")
            nc.tensor.transpose(pr_ps[:], phiT[:], ident[:])
            phirows = small.tile([BS, P], F32, tag="phirows")
            nc.scalar.copy(phirows[:], pr_ps[:])
            # G[p4, s, c] = phi_s[p4*32 + c]  (4 tiny SBUF->SBUF DMAs on the
            # Act HWDGE ring -- NOT gpsimd, whose queue must keep streaming
            # Linv without waiting on the encoder)
            g_f = wts.tile([4, BS, NBLK], F32)
            for p4 in range(4):
                nc.scalar.dma_start(out=g_f[p4:p4 + 1, :, :],
                                    in_=phirows[:, p4 * NBLK:(p4 + 1) * NBLK])
            g_bf = wts.tile([4, BS, NBLK], BF16)
            nc.vector.tensor_copy(g_bf[:], g_f[:])
            # phipat[p, s, c] = phi_s[(p%4)*32 + c] = E^T @ G
            pp_ps = qps.tile([128, BS * NBLK], F32, tag="qt", name="pp_ps")
            nc.tensor.matmul(pp_ps[:], e_rep[:], g_bf[:].rearrange("x s c -> x (s c)"),
                             start=True, stop=True)
            phipat = small.tile([128, BS, NBLK], BF16, tag="phipat")
            nc.scalar.copy(phipat[:], pp_ps[:].rearrange("p (s c) -> p s c", s=BS))

            # ---------- per-sample prep: R tiles + Q^T ----------
            r_tiles = []
            r32_tiles = {}
            qt_tiles = []
            for s in range(BS):
                rt = const.tile([128, NBLK, NBLK], BF16, name=f"r_{s}")
                nc.vector.tensor_tensor(
                    rt[:],
                    phipat[:, s, :][:, :, None].broadcast_to([128, NBLK, NBLK]),
                    mask[:, None, :].broadcast_to([128, NBLK, NBLK]),
                    op=ALU.mult)
                r_tiles.append(rt)
                qt_ps = qps.tile([P, ZU], F32, tag="qt", name=f"qt_ps_{s}")
                nc.tensor.transpose(qt_ps[:], qall[:, s, :], ident[:ZU, :ZU])
                qt = small.tile([P, ZU], BF16, tag="qt_sb", bufs=8, name=f"qt_{s}")
                nc.scalar.copy(qt[:], qt_ps[:])
                qt_tiles.append(qt)

            # ---------- main chunk pipeline ----------
            mscols = const.tile([ZU, BS, 2], F32)   # [zu, s, (sig|mu)]

            for s in range(BS):
                wall = small.tile([P, ZU], BF16, tag="wall", bufs=3, name=f"wall_{s}")
                for h in range(NCHUNK):
                    g = s * NCHUNK + h
                    if g + LOOKAHEAD < BS * NCHUNK:
                        load_chunk(g + LOOKAHEAD)
                    tf, is_f32 = chunk_tiles[g]
                    rh = r32_tiles[s] if is_f32 else r_tiles[s]
                    wp = wpsp.tile([P, CZU], F32, tag="w", name=f"wp_{g}")
                    for c in range(NBLK):
                        nc.tensor.matmul(wp[:], tf[:, c * 128:(c + 1) * 128],
                                         rh[:, c, :],
                                         start=(c == 0), stop=(c == NBLK - 1))
                    if h % 2 == 0:
                        nc.vector.tensor_copy(wall[:, h * CZU:(h + 1) * CZU], wp[:])
                    else:
                        nc.scalar.copy(wall[:, h * CZU:(h + 1) * CZU], wp[:])

                # pr = [ W*phi | W*qT ] -> ones-matmuls -> sig, mu columns
                pr = small.tile([P, 2 * ZU], BF16, tag="pr", name=f"pr_{s}")
                nc.vector.tensor_tensor(
                    pr[:, 0:ZU], wall[:],
                    phiT_bf[:, s:s + 1].broadcast_to([P, ZU]), op=ALU.mult)
                nc.vector.tensor_tensor(pr[:, ZU:2 * ZU], wall[:], qt_tiles[s][:],
                                        op=ALU.mult)
                ms_ps = rps.tile([ZU, 2], F32, tag="rows", name=f"ms_{s}")
                nc.tensor.matmul(ms_ps[:, 0:1], pr[:, 0:ZU], ones_bf[:],
                                 start=True, stop=True)
                nc.tensor.matmul(ms_ps[:, 1:2], pr[:, ZU:2 * ZU], ones_bf[:],
                                 start=True, stop=True)
                nc.scalar.copy(mscols[:, s, :], ms_ps[:])

            # ---------- finalize ----------
            cov64 = const.tile([ZU, BS], F32)
            nc.vector.tensor_scalar_add(cov64[:], mscols[:, :, 0], 1.0)
            nc.vector.tensor_scalar_mul(cov64[:], cov64[:], sigfac64[:])
            mu_t_ps = encps.tile([BS, ZU], F32, tag="t", name="mu_t_ps")
            nc.tensor.transpose(mu_t_ps[:], mscols[:, :, 1], ident[:ZU, :ZU])
            mu_t = small.tile([BS, ZU], F32, tag="mu_t")
            nc.scalar.copy(mu_t[:], mu_t_ps[:])
            nc.sync.dma_start(out=mu_d[:].rearrange("s z u one -> s (z u one)"), in_=mu_t[:])
            cov_t_ps = encps.tile([BS, ZU], F32, tag="t", name="cov_t_ps")
            nc.tensor.transpose(cov_t_ps[:], cov64[:], ident[:ZU, :ZU])
            cov_t = small.tile([BS, ZU], F32, tag="cov_t")
            nc.scalar.copy(cov_t[:], cov_t_ps[:])
            nc.sync.dma_start(out=cov_d[:].rearrange("s z u -> s (z u)"), in_=cov_t[:])

    nc.compile()
    return nc


_NC = None


def _get_nc():
    global _NC
    if _NC is None:
        _NC = build_nc()
    return _NC


def _in_maps(inputs):
    f = {k: np.ascontiguousarray(np.asarray(v, dtype=np.float32)) for k, v in inputs.items()}
    maps = []
    for c in range(NCORES):
        sl = slice(c * BS, (c + 1) * BS)
        maps.append({
            "x": f["x"][sl],
            "Linv": f["Linv"][sl],
            "Q": f["Q"][sl],
            "W1": f["W1"], "b1": f["b1"],
            "W2": f["W2"], "b2": f["b2"],
            "W3": f["W3"], "b3": f["b3"],
            "W4": f["W4"], "b4": f["b4"],
            "logSigEps": f["logSigEps"],
        })
    return maps


def kernel(**inputs):
    from concourse.bass_utils import run_bass_kernel_spmd

    nc = _get_nc()
    maps = _in_maps(inputs)
    res = run_bass_kernel_spmd(nc, maps, core_ids=list(range(NCORES)))
    mu = np.concatenate([np.asarray(res.results[i]["mu_out"]) for i in range(NCORES)], axis=0)
    cov = np.concatenate([np.asarray(res.results[i]["cov_out"]) for i in range(NCORES)], axis=0)
    return mu.astype(np.float32), cov.astype(np.float32)
